# revision 2
# baseline (speedup 1.0000x reference)
"""Multi-head self-attention (12 heads, d=64, seq=1024, batch=8) on 8 trn2 NeuronCores.

Sharding: data-parallel over batch — core b computes batch element b end-to-end
(attention is independent per batch element; no collectives needed).

Per-core dataflow (all matmuls in fp32r = full-rate fp32 streaming):
  inputs (host-prepped): xT [768,1024] (= x[b].T), Wq/Wk/Wv [768,768]
    (deinterleaved from W_qkv's (h,v,3) column order; Wq pre-scaled by 1/8),
    Wm [768,768].
  V   = x @ Wv               -> V_sb [128p x (12h,65)] tiles, col 64 = 1.0 (rowsum trick)
  QT  = Wq.T-chunks @ xT     -> per head-pair tile [128f, 1024p]  (f-major!)
  KT  likewise
  S^T = KT_h.T-chunk @ QT_h  -> PSUM [128j, 512i] per (pair: A rows 0-63, B rows 64-127
                                run concurrently on PE via row-group tiling)
  P^T = exp(S^T)             -> one ACT op [128, 1024] spanning both heads' banks
  O^T = V_ext.T @ P^T        -> PSUM [65, 512] accum over j; row 64 = softmax denom r
  norm: rinv = approx_recip(r); bcast via K=1 ones-matmul; O^T *= bcast (DVE)
  outT pairs [128f, 1024i] assembled (B half placed via SBUF->SBUF DMA)
  y   = outT.T-chunks @ Wm   -> [1024, 768] -> DRAM
"""

import numpy as np

H = 12          # heads
VD = 64         # head dim
D = 768         # token dim
S = 1024        # sequence length
B = 8           # batch
NCORES = 8
NPAIR = 6       # head pairs
TC = D // 128   # 6 contraction chunks of 128

_CACHE: dict = {}


def _build_nc():
    import concourse.bass as bass
    import concourse.mybir as mybir
    import concourse.tile as tile

    f32 = mybir.dt.float32
    f32r = mybir.dt.float32r
    EXP = mybir.ActivationFunctionType.Exp
    MULT = mybir.AluOpType.mult

    nc = bass.Bass()
    xT_d = nc.dram_tensor("xT", [D, S], f32, kind="ExternalInput")
    Wq_d = nc.dram_tensor("Wq", [D, D], f32, kind="ExternalInput")
    Wk_d = nc.dram_tensor("Wk", [D, D], f32, kind="ExternalInput")
    Wv_d = nc.dram_tensor("Wv", [D, D], f32, kind="ExternalInput")
    Wm_d = nc.dram_tensor("Wm", [D, D], f32, kind="ExternalInput")
    y_d = nc.dram_tensor("y", [S, D], f32, kind="ExternalOutput")

    with tile.TileContext(nc) as tc:
        with (
            tc.tile_pool(name="const", bufs=1) as constp,
            tc.tile_pool(name="wts", bufs=2) as wtsp,
            tc.tile_pool(name="qk", bufs=2) as qkp,
            tc.tile_pool(name="exps", bufs=3) as expp,
            tc.tile_pool(name="oT", bufs=4) as oTp,
            tc.tile_pool(name="misc", bufs=2) as miscp,
            tc.tile_pool(name="ps1", bufs=2, space="PSUM") as ps1,     # 2 banks
            tc.tile_pool(name="ps_s", bufs=2, space="PSUM") as ps_s,   # 4 banks
            tc.tile_pool(name="ps_o", bufs=2, space="PSUM") as ps_o,   # 2 banks
        ):
            # ---------------- load persistent tensors ----------------
            xT_sb = []
            for c in range(TC):
                t = constp.tile([128, S], f32, name=f"xT{c}", tag=f"xT{c}")
                nc.sync.dma_start(out=t, in_=xT_d[128 * c:128 * (c + 1), :])
                xT_sb.append(t)
            Wv_sb = []
            for c in range(TC):
                t = constp.tile([128, D], f32, name=f"Wv{c}", tag=f"Wv{c}")
                nc.sync.dma_start(out=t, in_=Wv_d[128 * c:128 * (c + 1), :])
                Wv_sb.append(t)
            Wm_sb = []
            for c in range(TC):
                t = constp.tile([128, D], f32, name=f"Wm{c}", tag=f"Wm{c}")
                nc.sync.dma_start(out=t, in_=Wm_d[128 * c:128 * (c + 1), :])
                Wm_sb.append(t)

            ones_sb = constp.tile([1, VD], f32, name="ones", tag="ones")
            nc.vector.memset(ones_sb, 1.0)

            # ---------------- V = x @ Wv, laid out [128p, 12h, 65] with ones col ----
            V_sb = []
            for j in range(8):
                vt = constp.tile([128, H, VD + 1], f32, name=f"V{j}", tag=f"V{j}")
                nc.vector.memset(vt[:, :, VD], 1.0)
                for half in range(2):
                    ps = ps1.tile([128, 512], f32, name="vps", tag="ps1")
                    for c in range(TC):
                        nc.tensor.matmul(
                            ps[:, 0:384],
                            lhsT=xT_sb[c][:, 128 * j:128 * (j + 1)].bitcast(f32r),
                            rhs=Wv_sb[c][:, 384 * half:384 * (half + 1)].bitcast(f32r),
                            start=(c == 0), stop=(c == TC - 1),
                        )
                    nc.vector.tensor_copy(
                        out=vt[:, 6 * half:6 * half + 6, 0:VD],
                        in_=ps[:, 0:384].rearrange("p (h d) -> p h d", h=6),
                    )
                V_sb.append(vt)

            # ---------------- per head-pair: QT/KT then attention ----------------
            outT_sb = []
            for e in range(NPAIR):
                wq = wtsp.tile([128, TC, 128], f32, name="wq", tag="wq")
                wk = wtsp.tile([128, TC, 128], f32, name="wk", tag="wk")
                for c in range(TC):
                    nc.sync.dma_start(
                        out=wq[:, c, :],
                        in_=Wq_d[128 * c:128 * (c + 1), 128 * e:128 * (e + 1)])
                    nc.sync.dma_start(
                        out=wk[:, c, :],
                        in_=Wk_d[128 * c:128 * (c + 1), 128 * e:128 * (e + 1)])

                qt = qkp.tile([128, S], f32, name="qt", tag="qt")
                kt = qkp.tile([128, S], f32, name="kt", tag="kt")
                for half in range(2):
                    for w, dst in ((wq, qt), (wk, kt)):
                        ps = ps1.tile([128, 512], f32, name="qkps", tag="ps1")
                        for c in range(TC):
                            nc.tensor.matmul(
                                ps,
                                lhsT=w[:, c, :].bitcast(f32r),
                                rhs=xT_sb[c][:, 512 * half:512 * (half + 1)].bitcast(f32r),
                                start=(c == 0), stop=(c == TC - 1),
                            )
                        nc.vector.tensor_copy(
                            out=dst[:, 512 * half:512 * (half + 1)], in_=ps)

                # attention for heads (2e, 2e+1)
                oa = oTp.tile([VD + 1, S], f32, name="oa", tag="oT")
                ob = oTp.tile([VD + 1, S], f32, name="ob", tag="oT")
                for half in range(2):
                    opsA = ps_o.tile([VD + 1, 512], f32, name="opsA", tag="ps_o")
                    opsB = ps_o.tile([VD + 1, 512], f32, name="opsB", tag="ps_o")
                    for j in range(8):
                        sps = ps_s.tile([128, 1024], f32, name="sps", tag="ps_s")
                        nc.tensor.matmul(
                            sps[:, 0:512],
                            lhsT=kt[0:64, 128 * j:128 * (j + 1)].bitcast(f32r),
                            rhs=qt[0:64, 512 * half:512 * (half + 1)].bitcast(f32r),
                            start=True, stop=True,
                        )
                        nc.tensor.matmul(
                            sps[:, 512:1024],
                            lhsT=kt[64:128, 128 * j:128 * (j + 1)].bitcast(f32r),
                            rhs=qt[64:128, 512 * half:512 * (half + 1)].bitcast(f32r),
                            start=True, stop=True,
                        )
                        ex = expp.tile([128, 1024], f32, name="ex", tag="ex")
                        nc.scalar.activation(ex, sps, EXP)
                        nc.tensor.matmul(
                            opsA,
                            lhsT=V_sb[j][:, 2 * e, :].bitcast(f32r),
                            rhs=ex[:, 0:512].bitcast(f32r),
                            start=(j == 0), stop=(j == 7),
                        )
                        nc.tensor.matmul(
                            opsB,
                            lhsT=V_sb[j][:, 2 * e + 1, :].bitcast(f32r),
                            rhs=ex[:, 512:1024].bitcast(f32r),
                            start=(j == 0), stop=(j == 7),
                        )
                    nc.vector.tensor_copy(
                        out=oa[:, 512 * half:512 * (half + 1)], in_=opsA)
                    nc.vector.tensor_copy(
                        out=ob[:, 512 * half:512 * (half + 1)], in_=opsB)

                # normalize: rinv then broadcast-multiply
                rr = miscp.tile([2, S], f32, name="rr", tag="rr")
                nc.sync.dma_start(out=rr[0:1, :], in_=oa[VD:VD + 1, :])
                nc.sync.dma_start(out=rr[1:2, :], in_=ob[VD:VD + 1, :])
                ri = miscp.tile([2, S], f32, name="ri", tag="ri")
                nc.vector.reciprocal_approx_fast(out=ri, in_=rr)
                # matmul rhs must sit at base partition 0 — bounce row 1 down
                riB = miscp.tile([1, S], f32, name="riB", tag="riB")
                nc.sync.dma_start(out=riB, in_=ri[1:2, :])

                ot = constp.tile([128, S], f32, name=f"outT{e}", tag=f"outT{e}")
                for idx, osb in ((0, oa), (1, ob)):
                    rsrc = ri[0:1, :] if idx == 0 else riB
                    for half in range(2):
                        bps = ps1.tile([VD, 512], f32, name="bps", tag="ps1")
                        nc.tensor.matmul(
                            bps,
                            lhsT=ones_sb.bitcast(f32r),
                            rhs=rsrc[:, 512 * half:512 * (half + 1)].bitcast(f32r),
                            start=True, stop=True,
                        )
                        if idx == 0:
                            # head A lands on partitions 0-63 directly
                            nc.vector.tensor_tensor(
                                ot[0:VD, 512 * half:512 * (half + 1)],
                                osb[0:VD, 512 * half:512 * (half + 1)],
                                bps, MULT)
                        else:
                            # head B: partitions 0-63 -> 64-127 via SBUF DMA
                            nt = miscp.tile([VD, 512], f32, name="nt", tag="nt")
                            nc.vector.tensor_tensor(
                                nt,
                                osb[0:VD, 512 * half:512 * (half + 1)],
                                bps, MULT)
                            nc.sync.dma_start(
                                out=ot[VD:128, 512 * half:512 * (half + 1)], in_=nt)
                outT_sb.append(ot)

            # ---------------- merge: y = outT.T @ Wm ----------------
            for i in range(8):
                ysb = miscp.tile([128, D], f32, name="ysb", tag="ysb")
                for half in range(2):
                    yps = ps1.tile([128, 512], f32, name="yps", tag="ps1")
                    for e in range(NPAIR):
                        nc.tensor.matmul(
                            yps[:, 0:384],
                            lhsT=outT_sb[e][:, 128 * i:128 * (i + 1)].bitcast(f32r),
                            rhs=Wm_sb[e][:, 384 * half:384 * (half + 1)].bitcast(f32r),
                            start=(e == 0), stop=(e == NPAIR - 1),
                        )
                    nc.vector.tensor_copy(
                        out=ysb[:, 384 * half:384 * (half + 1)], in_=yps[:, 0:384])
                nc.sync.dma_start(out=y_d[128 * i:128 * (i + 1), :], in_=ysb)

    return nc


def _prep_inputs(x, W_qkv, W_merge):
    x = np.asarray(x, dtype=np.float32)
    Wf = np.asarray(W_qkv, dtype=np.float32).reshape(D, H, VD, 3)
    Wq = np.ascontiguousarray(Wf[..., 0].reshape(D, H * VD) * (VD ** -0.5))
    Wk = np.ascontiguousarray(Wf[..., 1].reshape(D, H * VD))
    Wv = np.ascontiguousarray(Wf[..., 2].reshape(D, H * VD))
    Wm = np.ascontiguousarray(np.asarray(W_merge, dtype=np.float32))
    in_maps = [
        {"xT": np.ascontiguousarray(x[b].T), "Wq": Wq, "Wk": Wk, "Wv": Wv, "Wm": Wm}
        for b in range(B)
    ]
    return in_maps


def kernel(x, W_qkv, W_merge):
    import sys
    for p in ("/opt/trn_rl_repo", "/opt/pypackages"):
        if p not in sys.path:
            sys.path.append(p)
    from concourse.bass_utils import run_bass_kernel_spmd

    nc = _CACHE.get("nc")
    if nc is None:
        nc = _build_nc()
        _CACHE["nc"] = nc
    in_maps = _prep_inputs(x, W_qkv, W_merge)
    res = run_bass_kernel_spmd(nc, in_maps, core_ids=list(range(NCORES)))
    return np.stack([res.results[b]["y"] for b in range(B)], axis=0)


# revision 24
# speedup vs baseline: 1.3064x; 1.3064x over previous
"""Multi-head self-attention (12 heads, d=64, seq=1024, batch=8) on 8 trn2 NeuronCores.

Sharding: data-parallel over batch — core b computes batch element b end-to-end
(attention is independent per batch element; no collectives needed).

Per-core dataflow (all matmuls in fp32r = full-rate fp32 streaming):
  inputs (host-prepped): xT [768,1024] (= x[b].T), Wq/Wk/Wv [768,768]
    (deinterleaved from W_qkv's (h,v,3) column order; Wq pre-scaled by 1/8),
    Wm [768,768].
  V   = x @ Wv               -> V_sb [128p x (12h,65)] tiles, col 64 = 1.0 (rowsum trick)
  QT  = Wq.T-chunks @ xT     -> per head-pair tile [128f, 1024p]  (f-major!)
  KT  likewise
  S^T = KT_h.T-chunk @ QT_h  -> PSUM [128j, 512i] per (pair: A rows 0-63, B rows 64-127
                                run concurrently on PE via row-group tiling)
  P^T = exp(S^T)             -> one ACT op [128, 1024] spanning both heads' banks
  O^T = V_ext.T @ P^T        -> PSUM [65, 512] accum over j; row 64 = softmax denom r
  norm: rinv = approx_recip(r); bcast via K=1 ones-matmul; O^T *= bcast (DVE)
  outT pairs [128f, 1024i] assembled (B half placed via SBUF->SBUF DMA)
  y   = outT.T-chunks @ Wm   -> [1024, 768] -> DRAM
"""

import numpy as np

H = 12          # heads
VD = 64         # head dim
D = 768         # token dim
S = 1024        # sequence length
B = 8           # batch
NCORES = 8
NPAIR = 6       # head pairs
TC = D // 128   # 6 contraction chunks of 128

_CACHE: dict = {}


def _build_nc(expp_bufs=3, wts_bufs=2, qk_bufs=2, oT_bufs=4, misc_bufs=2,
              skip_norm=False, use_f32r=True, stop_stage="full",
              use_fast_recip=True):
    import concourse.bass as bass
    import concourse.mybir as mybir
    import concourse.tile as tile

    f32 = mybir.dt.float32
    f32r = mybir.dt.float32r if use_f32r else mybir.dt.float32
    EXP = mybir.ActivationFunctionType.Exp
    MULT = mybir.AluOpType.mult

    nc = bass.Bass()
    xT_d = nc.dram_tensor("xT", [D, S], f32r, kind="ExternalInput")
    Wq_d = nc.dram_tensor("Wq", [D, D], f32r, kind="ExternalInput")
    Wk_d = nc.dram_tensor("Wk", [D, D], f32r, kind="ExternalInput")
    Wv_d = nc.dram_tensor("Wv", [D, D], f32r, kind="ExternalInput")
    Wm_d = nc.dram_tensor("Wm", [D, D], f32r, kind="ExternalInput")
    y_d = nc.dram_tensor("y", [S, D], f32, kind="ExternalOutput")

    with tile.TileContext(nc) as tc:
        with (
            tc.tile_pool(name="const", bufs=1) as constp,
            tc.tile_pool(name="wts", bufs=wts_bufs) as wtsp,
            tc.tile_pool(name="qk", bufs=qk_bufs) as qkp,
            tc.tile_pool(name="exps", bufs=expp_bufs) as expp,
            tc.tile_pool(name="oT", bufs=oT_bufs) as oTp,
            tc.tile_pool(name="misc", bufs=misc_bufs) as miscp,
            tc.tile_pool(name="ps1", bufs=2, space="PSUM") as ps1,     # 2 banks
            tc.tile_pool(name="ps_s", bufs=2, space="PSUM") as ps_s,   # 4 banks
            tc.tile_pool(name="ps_o", bufs=2, space="PSUM") as ps_o,   # 2 banks
        ):
            # ---------------- load persistent tensors ----------------
            xT_sb = []
            for c in range(TC):
                t = constp.tile([128, S], f32r, name=f"xT{c}", tag=f"xT{c}")
                nc.sync.dma_start(out=t, in_=xT_d[128 * c:128 * (c + 1), :])
                xT_sb.append(t)
            Wv_sb = []
            for c in range(TC):
                t = constp.tile([128, D], f32r, name=f"Wv{c}", tag=f"Wv{c}")
                nc.sync.dma_start(out=t, in_=Wv_d[128 * c:128 * (c + 1), :])
                Wv_sb.append(t)

            # ones column vector for the K=1 broadcast matmul; spans partitions
            # 0..64 so row 64 can serve as a base-partition-64 lhsT
            ones_sb = constp.tile([VD + 1, VD], f32r, name="ones", tag="ones")
            nc.vector.memset(ones_sb, 1.0)

            # QT/KT for one head pair: [128f, 1024p], A rows 0-63 / B rows 64-127
            def emit_qtkt(e):
                # single DMA per weight tile: one dma-queue semaphore, so the
                # consuming matmuls stay under the per-instruction wait budget
                wq = wtsp.tile([128, TC, 128], f32r, name="wq", tag="wq")
                wk = wtsp.tile([128, TC, 128], f32r, name="wk", tag="wk")
                nc.sync.dma_start(
                    out=wq,
                    in_=Wq_d[:, 128 * e:128 * (e + 1)].rearrange(
                        "(c p) f -> p c f", p=128))
                nc.sync.dma_start(
                    out=wk,
                    in_=Wk_d[:, 128 * e:128 * (e + 1)].rearrange(
                        "(c p) f -> p c f", p=128))
                qt = qkp.tile([128, S], f32r, name="qt", tag="qt")
                kt = qkp.tile([128, S], f32r, name="kt", tag="kt")
                for half in range(2):
                    for w, dst in ((wq, qt), (wk, kt)):
                        ps = ps1.tile([128, 512], f32, name="qkps", tag="ps1")
                        for c in range(TC):
                            nc.tensor.matmul(
                                ps,
                                lhsT=w[:, c, :],
                                rhs=xT_sb[c][:, 512 * half:512 * (half + 1)],
                                start=(c == 0), stop=(c == TC - 1),
                            )
                        nc.vector.tensor_copy(
                            out=dst[:, 512 * half:512 * (half + 1)], in_=ps)
                return qt, kt

            # pair 0's QT/KT first so its scores/exp start before V finishes
            next_qtkt = emit_qtkt(0)

            # ---------------- V = x @ Wv, laid out [128p, 12h, 65] with ones col ----
            V_sb = []
            for j in range(8):
                vt = constp.tile([128, H, VD + 1], f32r, name=f"V{j}", tag=f"V{j}")
                nc.vector.memset(vt[:, :, VD], 1.0)
                for half in range(2):
                    ps = ps1.tile([128, 512], f32, name="vps", tag="ps1")
                    for c in range(TC):
                        nc.tensor.matmul(
                            ps[:, 0:384],
                            lhsT=xT_sb[c][:, 128 * j:128 * (j + 1)],
                            rhs=Wv_sb[c][:, 384 * half:384 * (half + 1)],
                            start=(c == 0), stop=(c == TC - 1),
                        )
                    nc.vector.tensor_copy(
                        out=vt[:, 6 * half:6 * half + 6, 0:VD],
                        in_=ps[:, 0:384].rearrange("p (h d) -> p h d", h=6),
                    )
                V_sb.append(vt)

            # ---------------- per head-pair: QT/KT then attention ----------------
            outT_sb = [None] * NPAIR
            Wm_sb = []
            pend_norm = []

            def emit_norm(e, oa, ob):
                if skip_norm:
                    ot = constp.tile([128, S], f32r, name=f"outT{e}", tag=f"outT{e}")
                    nc.vector.tensor_copy(out=ot[0:VD, :], in_=oa[0:VD, :])
                    nc.sync.dma_start(out=ot[VD:128, :], in_=ob[0:VD, :])
                    outT_sb[e] = ot
                    return
                # 1/rowsum in place on the r rows (partition 64 is matmul-legal).
                # raw _custom_dve: the wrapper asserts fp32 but these tiles are
                # fp32r (same bit layout) so the exponent-flip seed still works
                if use_fast_recip:
                    from concourse.dve_ops import (
                        RECIP_APPROX_FAST_CONSTS as _RC,
                        RECIPROCAL_APPROX_FAST as _RF,
                    )
                    for _o in (oa, ob):
                        nc.vector._custom_dve(
                            _RF, out=_o[VD:VD + 1, :], in0=_o[VD:VD + 1, :],
                            s0=_RC["s0"], s1=_RC["s1"], imm2=_RC["imm2"])
                else:
                    with nc.allow_low_precision(reason="f32r view of softmax denom"):
                        for _o in (oa, ob):
                            nc.vector.reciprocal(
                                out=_o[VD:VD + 1, :], in_=_o[VD:VD + 1, :])
                ot = constp.tile([128, S], f32r, name=f"outT{e}", tag=f"outT{e}")
                nt = miscp.tile([VD, S], f32r, name="nt", tag="nt")
                for idx, osb in ((0, oa), (1, ob)):
                    for half in range(2):
                        hs = slice(512 * half, 512 * (half + 1))
                        bps = ps1.tile([VD, 512], f32, name="bps", tag="ps1")
                        nc.tensor.matmul(
                            bps,
                            lhsT=ones_sb[VD:VD + 1, :],
                            rhs=osb[VD:VD + 1, hs],
                            start=True, stop=True,
                        )
                        if idx == 0:
                            # head A lands on partitions 0-63 directly
                            nc.vector.tensor_tensor(
                                ot[0:VD, hs], osb[0:VD, hs], bps, MULT)
                        else:
                            # head B: partitions 0-63 -> 64-127 via SBUF DMA
                            nc.vector.tensor_tensor(nt[:, hs], osb[0:VD, hs], bps, MULT)
                # single DMA so the merge matmul waits on one dma semaphore
                nc.sync.dma_start(out=ot[VD:128, :], in_=nt)
                outT_sb[e] = ot

            for e in range(NPAIR):
                qt, kt = next_qtkt
                if stop_stage == "qkt":
                    if e + 1 < NPAIR:
                        next_qtkt = emit_qtkt(e + 1)
                    continue

                # attention for heads (2e, 2e+1)
                oa = oTp.tile([VD + 1, S], f32r, name="oa", tag="oT")
                ob = oTp.tile([VD + 1, S], f32r, name="ob", tag="oT")
                for half in range(2):
                    opsA = ps_o.tile([VD + 1, 512], f32, name="opsA", tag="ps_o")
                    opsB = ps_o.tile([VD + 1, 512], f32, name="opsB", tag="ps_o")
                    for j in range(8):
                        sps = ps_s.tile([128, 1024], f32, name="sps", tag="ps_s")
                        nc.tensor.matmul(
                            sps[:, 0:512],
                            lhsT=kt[0:64, 128 * j:128 * (j + 1)],
                            rhs=qt[0:64, 512 * half:512 * (half + 1)],
                            start=True, stop=True,
                        )
                        nc.tensor.matmul(
                            sps[:, 512:1024],
                            lhsT=kt[64:128, 128 * j:128 * (j + 1)],
                            rhs=qt[64:128, 512 * half:512 * (half + 1)],
                            start=True, stop=True,
                        )
                        ex = expp.tile([128, 1024], f32r, name="ex", tag="ex")
                        nc.scalar.activation(ex, sps, EXP)
                        if stop_stage == "scores":
                            continue
                        nc.tensor.matmul(
                            opsA,
                            lhsT=V_sb[j][:, 2 * e, :],
                            rhs=ex[:, 0:512],
                            start=(j == 0), stop=(j == 7),
                        )
                        nc.tensor.matmul(
                            opsB,
                            lhsT=V_sb[j][:, 2 * e + 1, :],
                            rhs=ex[:, 512:1024],
                            start=(j == 0), stop=(j == 7),
                        )
                    if stop_stage != "scores":
                        nc.vector.tensor_copy(
                            out=oa[:, 512 * half:512 * (half + 1)], in_=opsA)
                        nc.vector.tensor_copy(
                            out=ob[:, 512 * half:512 * (half + 1)], in_=opsB)

                if e + 1 < NPAIR:
                    next_qtkt = emit_qtkt(e + 1)
                if e == 2:
                    # merge weights — off the startup DMA path, early enough
                    # to be resident before the merge begins
                    for c in range(TC):
                        t = constp.tile([128, D], f32r, name=f"Wm{c}", tag=f"Wm{c}")
                        nc.sync.dma_start(
                            out=t, in_=Wm_d[128 * c:128 * (c + 1), :])
                        Wm_sb.append(t)
                if stop_stage in ("scores",):
                    continue
                pend_norm.append((e, oa, ob))
                # defer normalization one pair so it overlaps the next pair's
                # attention instead of convoying ahead of it on DVE/PE
                if len(pend_norm) > 1:
                    emit_norm(*pend_norm.pop(0))

            while pend_norm:
                emit_norm(*pend_norm.pop(0))

            merge_on = stop_stage in ("full",)

            # ---------------- merge: y = outT.T @ Wm ----------------
            for i in range(8 if merge_on else 0):
                ysb = miscp.tile([128, D], f32, name="ysb", tag="ysb")
                for half in range(2):
                    yps = ps1.tile([128, 512], f32, name="yps", tag="ps1")
                    for e in range(NPAIR):
                        nc.tensor.matmul(
                            yps[:, 0:384],
                            lhsT=outT_sb[e][:, 128 * i:128 * (i + 1)],
                            rhs=Wm_sb[e][:, 384 * half:384 * (half + 1)],
                            start=(e == 0), stop=(e == NPAIR - 1),
                        )
                    # scalar engine: idle once the last exp is done
                    nc.scalar.copy(
                        out=ysb[:, 384 * half:384 * (half + 1)], in_=yps[:, 0:384])
                nc.sync.dma_start(out=y_d[128 * i:128 * (i + 1), :], in_=ysb)

    return nc


def _prep_inputs(x, W_qkv, W_merge):
    x = np.asarray(x, dtype=np.float32)
    Wf = np.asarray(W_qkv, dtype=np.float32).reshape(D, H, VD, 3)
    Wq = np.ascontiguousarray(Wf[..., 0].reshape(D, H * VD) * (VD ** -0.5))
    Wk = np.ascontiguousarray(Wf[..., 1].reshape(D, H * VD))
    Wv = np.ascontiguousarray(Wf[..., 2].reshape(D, H * VD))
    Wm = np.ascontiguousarray(np.asarray(W_merge, dtype=np.float32))
    in_maps = [
        {"xT": np.ascontiguousarray(x[b].T), "Wq": Wq, "Wk": Wk, "Wv": Wv, "Wm": Wm}
        for b in range(B)
    ]
    return in_maps


_VARIANTS = [
    dict(),
    dict(use_fast_recip=False),
    dict(expp_bufs=4, oT_bufs=6),
    dict(expp_bufs=4, oT_bufs=6, use_fast_recip=False),
    dict(qk_bufs=3, misc_bufs=3, expp_bufs=2, oT_bufs=2),
    dict(qk_bufs=3, misc_bufs=3, expp_bufs=2, oT_bufs=2, use_fast_recip=False),
    dict(wts_bufs=3, expp_bufs=2),
    dict(wts_bufs=3, expp_bufs=2, use_fast_recip=False),
]


def kernel(x, W_qkv, W_merge):
    import sys
    for p in ("/opt/trn_rl_repo", "/opt/pypackages"):
        if p not in sys.path:
            sys.path.append(p)
    from concourse.bass_utils import run_bass_kernel_spmd

    in_maps = _prep_inputs(x, W_qkv, W_merge)
    last_err = None
    if _CACHE.get("nc") is not None:
        try:
            res = run_bass_kernel_spmd(
                _CACHE["nc"], in_maps, core_ids=list(range(NCORES)))
            return np.stack([res.results[b]["y"] for b in range(B)], axis=0)
        except Exception as ex:  # fall through to the variant ladder
            last_err = ex
            _CACHE["nc"] = None
    # walrus wait-placement is schedule-dependent and occasionally rejects a
    # build; different buffer configs reshuffle the schedule, so walk a small
    # ladder until one compiles and executes
    for cfg in _VARIANTS:
        try:
            nc = _build_nc(**cfg)
            res = run_bass_kernel_spmd(nc, in_maps, core_ids=list(range(NCORES)))
            _CACHE["nc"] = nc
            return np.stack([res.results[b]["y"] for b in range(B)], axis=0)
        except Exception as ex:
            last_err = ex
    raise last_err
